# revision 5
# baseline (speedup 1.0000x reference)
"""MixerHead kernel for 8 trn2 NeuronCores (Bass/Tile, fp32r matmuls).

Math (reference):
  proj[b,h,l,e]  = sum_d x[b,l,d] Wp[h,e,d] + bp[h,e]
  mixed[b,h,f,e] = sum_{l<=f} Wc[h,f,l] proj[b,h,l,e] + bc[h,f]
  out[b,f,j]     = sum_{h,e} mixed[b,h,f,e] Wo[j, h*E+e] + bo[j]

Sharding: core c = (batch b = c//2, head-pair hp = c%2 -> heads {2hp, 2hp+1}).
Each core computes the bias-free linear part for its (batch, 2 heads) and
writes a partial [L, D] output; host sums the two partials per batch and adds
all bias contributions (folded into a single [L, D] matrix analytically).

Device layout chain (every matmul is out = lhsT.T @ rhs, contraction on the
partition dim, fp32r so the PE runs at full rate):
  phase1: proj[l,e]    lhsT = xT[d, l-tile]          rhs = WpT[d, e(512)]
  phase2: mixedT[e,f]  lhsT = proj[l-tile, e-block]  rhs = WcT[l-tile, f-chunk]
          (WcT is pre-masked tril(Wc).T, packed on host so only lower-tri
           l-tiles are stored/loaded/computed)
  phase3: part[f,dout] lhsT = mixedT[e-blk, f-tile]  rhs = WoT[e-blk, dout]
"""

import sys

for _p in ("/opt/trn_rl_repo", "/root/.axon_site/_ro/trn_rl_repo"):
    if _p not in sys.path:
        sys.path.append(_p)

import numpy as np

from concourse import bacc, mybir, tile
from concourse.bass_utils import run_bass_kernel_spmd

B, L, D, H, E = 4, 2048, 1024, 4, 256
F32 = mybir.dt.float32
F32R = mybir.dt.float32r

LT = L // 128   # 16 l-tiles per batch
FC = 4          # f-chunks of 512
DT8 = D // 128  # 8 d-tiles
WC_PACK_COLS = sum((4 * c + 4) * 512 for c in range(FC))  # 20480

# Set by test harness: run with trace and record exec time.
TRACE = False
LAST_EXEC_NS = None

_cache = {}


def _build_program():
    if "nc" in _cache:
        return _cache["nc"]
    nc = bacc.Bacc("TRN2", target_bir_lowering=False, debug=False, num_devices=8)

    xT = nc.dram_tensor("xT", [D, L], F32R, kind="ExternalInput")
    wpT = nc.dram_tensor("wpT", [D, 2 * E], F32R, kind="ExternalInput")
    wc0 = nc.dram_tensor("wc0", [128, WC_PACK_COLS], F32R, kind="ExternalInput")
    wc1 = nc.dram_tensor("wc1", [128, WC_PACK_COLS], F32R, kind="ExternalInput")
    woT = nc.dram_tensor("woT", [2 * E, D], F32R, kind="ExternalInput")
    part = nc.dram_tensor("part", [L, D], F32, kind="ExternalOutput")
    wc_dram = [wc0, wc1]

    with tile.TileContext(nc) as tc:
        with (
            tc.tile_pool(name="wp", bufs=1) as wp_pool,
            tc.tile_pool(name="wo", bufs=1) as wo_pool,
            tc.tile_pool(name="xt", bufs=3) as x_pool,
            tc.tile_pool(name="wc", bufs=20) as wc_pool,
            tc.tile_pool(name="proj", bufs=1) as proj_pool,
            tc.tile_pool(name="mix", bufs=1) as mix_pool,
            tc.tile_pool(name="outs", bufs=4) as out_pool,
            tc.tile_pool(name="ps1", bufs=1, space="PSUM") as ps1_pool,
            tc.tile_pool(name="ps2", bufs=2, space="PSUM") as ps2_pool,
            tc.tile_pool(name="ps3", bufs=2, space="PSUM") as ps3_pool,
        ):
            # Resident weights.
            wp = []
            for d in range(DT8):
                t = wp_pool.tile([128, 2 * E], F32R, tag=f"wp{d}")
                nc.sync.dma_start(t[:], wpT[d * 128 : (d + 1) * 128, :])
                wp.append(t)
            wo = []
            for eb in range(4):
                t = wo_pool.tile([128, D], F32R, tag=f"wo{eb}")
                nc.sync.dma_start(t[:], woT[eb * 128 : (eb + 1) * 128, :])
                wo.append(t)

            # Phase 1: proj[l, e] for all 16 l-tiles (e = 2 heads * 256 = 512).
            proj = [None] * LT
            for lc in range(LT // 4):  # l-chunks of 512 (4 l-tiles)
                ps = [
                    ps1_pool.tile([128, 2 * E], F32, tag=f"ps1_{i}", name=f"ps1_{lc}_{i}")
                    for i in range(4)
                ]
                for d in range(DT8):
                    xt = x_pool.tile([128, 512], F32R)
                    nc.sync.dma_start(
                        xt[:], xT[d * 128 : (d + 1) * 128, lc * 512 : (lc + 1) * 512]
                    )
                    for i in range(4):
                        nc.tensor.matmul(
                            ps[i][:],
                            xt[:, i * 128 : (i + 1) * 128],
                            wp[d][:],
                            start=(d == 0),
                            stop=(d == DT8 - 1),
                        )
                for i in range(4):
                    lt = lc * 4 + i
                    pt = proj_pool.tile([128, 2 * E], F32R, tag=f"proj{lt}")
                    nc.vector.tensor_copy(pt[:], ps[i][:])
                    proj[lt] = pt

            # Phase 2: mixedT[e, f] per (e-block, f-chunk); causal => only
            # l-tiles 0..4c+3 contribute to f-chunk c (mask pre-applied in Wc).
            mix = [[None] * FC for _ in range(4)]
            wc_off = 0
            for c in range(FC):
                T = 4 * c + 4
                for hh in range(2):
                    wct = []
                    for t in range(T):
                        wt = wc_pool.tile(
                            [128, 512], F32R, tag="wcring", name=f"wc_{c}_{hh}_{t}"
                        )
                        nc.sync.dma_start(
                            wt[:],
                            wc_dram[hh][:, wc_off + t * 512 : wc_off + (t + 1) * 512],
                        )
                        wct.append(wt)
                    for eb in (2 * hh, 2 * hh + 1):
                        ps = ps2_pool.tile([128, 512], F32)
                        for t in range(T):
                            nc.tensor.matmul(
                                ps[:],
                                proj[t][:, eb * 128 : (eb + 1) * 128],
                                wct[t][:],
                                start=(t == 0),
                                stop=(t == T - 1),
                            )
                        mt = mix_pool.tile([128, 512], F32R, tag=f"m{eb}_{c}")
                        nc.vector.tensor_copy(mt[:], ps[:])
                        mix[eb][c] = mt
                wc_off += T * 512

            # Phase 3: partial out[f, dout] = sum_e mixedT[e, f] * WoT[e, dout].
            for ft in range(LT):
                c = ft // 4
                fi = ft % 4
                for dc in range(2):
                    ps = ps3_pool.tile([128, 512], F32)
                    for eb in range(4):
                        nc.tensor.matmul(
                            ps[:],
                            mix[eb][c][:, fi * 128 : (fi + 1) * 128],
                            wo[eb][:, dc * 512 : (dc + 1) * 512],
                            start=(eb == 0),
                            stop=(eb == 3),
                        )
                    ot = out_pool.tile([128, 512], F32)
                    nc.vector.tensor_copy(ot[:], ps[:])
                    nc.sync.dma_start(
                        part[ft * 128 : (ft + 1) * 128, dc * 512 : (dc + 1) * 512],
                        ot[:],
                    )

    nc.compile()
    _cache["nc"] = nc
    return nc


def _pack_wc_head(wc_h: np.ndarray) -> np.ndarray:
    """tril(Wc[h]) -> [128, 20480]: per f-chunk c, the l-tiles 0..4c+3 of
    WcT = tril(Wc).T laid out as [128 l-partitions, T*512 f-cols]."""
    m = np.tril(wc_h)  # [f, l]
    blocks = []
    for c in range(FC):
        T = 4 * c + 4
        sub = m[c * 512 : (c + 1) * 512, : T * 128]  # [512 f, T*128 l]
        subT = sub.T.reshape(T, 128, 512)  # [T, 128 l, 512 f]
        blocks.append(subT.transpose(1, 0, 2).reshape(128, T * 512))
    return np.ascontiguousarray(np.concatenate(blocks, axis=1), dtype=np.float32)


def kernel(x, Wp, bp, Wc, bc, Wo, bo):
    global LAST_EXEC_NS
    x = np.asarray(x, dtype=np.float32)
    Wp = np.asarray(Wp, dtype=np.float32)
    bp = np.asarray(bp, dtype=np.float32)
    Wc = np.asarray(Wc, dtype=np.float32)
    bc = np.asarray(bc, dtype=np.float32)
    Wo = np.asarray(Wo, dtype=np.float32)
    bo = np.asarray(bo, dtype=np.float32)

    nc = _build_program()

    WoT = np.ascontiguousarray(Wo.T)  # [din, dout]
    wc_packed = [_pack_wc_head(Wc[h]) for h in range(H)]
    wpT_pair = []
    woT_pair = []
    for hp in range(2):
        h0, h1 = 2 * hp, 2 * hp + 1
        wpT_pair.append(
            np.ascontiguousarray(
                np.concatenate([Wp[h0].T, Wp[h1].T], axis=1), dtype=np.float32
            )
        )
        woT_pair.append(
            np.ascontiguousarray(
                np.concatenate(
                    [WoT[h0 * E : (h0 + 1) * E], WoT[h1 * E : (h1 + 1) * E]], axis=0
                ),
                dtype=np.float32,
            )
        )

    in_maps = []
    for c in range(8):
        b, hp = c // 2, c % 2
        in_maps.append(
            {
                "xT": np.ascontiguousarray(x[b].T, dtype=np.float32),
                "wpT": wpT_pair[hp],
                "wc0": wc_packed[2 * hp],
                "wc1": wc_packed[2 * hp + 1],
                "woT": woT_pair[hp],
            }
        )

    res = run_bass_kernel_spmd(
        nc, in_maps, core_ids=list(range(8)), trace=TRACE
    )
    LAST_EXEC_NS = res.exec_time_ns

    # Host: fold all bias terms into one [L, D] matrix.
    # mixed bias = tril-rowsum(Wc)[h,f] * bp[h,e] + bc[h,f]; through Wo:
    rs = np.tril(Wc).sum(axis=2)  # [H, L]
    Wo_hE = Wo.reshape(D, H, E)
    V = np.einsum("he,jhe->hj", bp, Wo_hE)  # [H, D]
    WoSum = Wo_hE.sum(axis=2)  # [D, H]
    bias_total = rs.T @ V + bc.T @ WoSum.T + bo[None, :]  # [L, D]

    out = np.empty((B, L, D), dtype=np.float32)
    for b in range(B):
        out[b] = (
            res.results[2 * b]["part"] + res.results[2 * b + 1]["part"] + bias_total
        )
    return out
